# revision 1
# baseline (speedup 1.0000x reference)
"""Trainium2 Bass kernel: channel-attention MultiHeadAttention block.

Full (unsharded) inputs in, full output out. Internally: data-parallel over
batch B across 8 NeuronCores (1 batch each), with a tiny AllReduce for the
BatchNorm batch statistics.

Per-core math (batch b), all shapes [partition, free]:
  qsb/ksb/vsb   [65, 4096]   inputs + ones row (bias fold)
  wqe/wke/wve   [65, 512]    [W.T; bias]
  QT_m, KT_m    [128, 512]   projections transposed (m on partitions)
  scores        [128c, 512d] = sum_m QT[:,c-chunk].T @ KT      (4 psum tiles)
  attn          exp(scores/64) via ACT (+row sums)
  attnT         PE-transpose blocks fused with diag(1/rowsum)
  V             [128d, 4096m] natural layout
  X[cc]         [128c', 4096s'] = attn@V directly in post-permute BN layout,
                via stride-8 m-slices of V as the matmul stationary operand
  BN stats      row sums/sumsq -> AllReduce over 8 cores -> alpha/beta
  BN+leaky      in-place ACT Lrelu(scale=alpha, bias=beta)
  w1 + leaky    [512,512] conv, bias+leaky fused in ACT
  w2 + bias     [64,512] conv -> y [64, 4096]
"""

import sys

if "/opt/trn_rl_repo" not in sys.path:
    sys.path.insert(0, "/opt/trn_rl_repo")

import numpy as np

import concourse.bacc as bacc
import concourse.mybir as mybir
import concourse.tile as tile
from concourse import bass_utils

B = 8
C = 64
CN = 512
HW = 4096
NM = HW // 128   # 32 m-chunks
NCH = CN // 128  # 4 channel chunks
NS = HW // 512   # 8 free-dim slices
EPS = 1e-4
SLOPE = 0.01
INV_SCALE = 1.0 / 64.0      # 1/sqrt(HW)
INV_BHW = 1.0 / (B * HW)    # BN divisor

F32 = mybir.dt.float32
F32R = mybir.dt.float32r
AF = mybir.ActivationFunctionType
ALU = mybir.AluOpType
AX = mybir.AxisListType
RG = [[0, 1, 2, 3, 4, 5, 6, 7]]


def _r(ap):
    return ap.bitcast(F32R)


def _body(tc, nc, d, dbg=None):
    with (
        tc.tile_pool(name="consts", bufs=1) as consts,
        tc.tile_pool(name="small", bufs=1) as small,
        tc.tile_pool(name="atp", bufs=1) as atp,
        tc.tile_pool(name="vbuf", bufs=1) as vpool,
    ):
        # ---- weights / constants: packed tile, split DMA (proj part 1st)
        wpack = consts.tile([128, 4000], F32R, name="wpack", tag="wpack")
        nc.sync.dma_start(wpack[:, 0:1536], d["wpack"][:, 0:1536])
        nc.scalar.dma_start(wpack[:, 1536:4000], d["wpack"][:, 1536:4000])
        wqe = wpack[0:65, 0:512]
        wke = wpack[0:65, 512:1024]
        wve = wpack[0:65, 1024:1536]
        w1sb = [wpack[:, 1536 + 512 * cc:1536 + 512 * (cc + 1)]
                for cc in range(NCH)]
        w2sb = [wpack[:, 3584 + 64 * oc:3584 + 64 * (oc + 1)]
                for oc in range(NCH)]
        ident = wpack[:, 3840:3968]
        b1sb = wpack[:, 3968:3972].bitcast(F32)
        b2sb = wpack[0:64, 3972:3973].bitcast(F32)
        bngsb = wpack[:, 3976:3980].bitcast(F32)
        bnbsb = wpack[:, 3980:3984].bitcast(F32)

        # warmup collective: absorbs ncfw cold-start + inter-core skew early
        with tc.tile_pool(name="wudram", bufs=1, space="DRAM") as wud:
            dwin = wud.tile([128, 1], F32, name="dwin", tag="dwin")
            dwout = wud.tile([128, 1], F32, name="dwout", tag="dwout")
            wsrc = small.tile([128, 1], F32, name="wsrc", tag="wsrc")
            nc.gpsimd.memset(wsrc[:], 1.0)
            nc.gpsimd.dma_start(dwin[:], wsrc[:])
            nc.gpsimd.collective_compute(
                "AllReduce", ALU.add, replica_groups=RG,
                ins=[dwin.opt()], outs=[dwout.opt()])
            wdst = small.tile([128, 1], F32, name="wdst", tag="wdst")
            nc.gpsimd.dma_start(wdst[:], dwout[:])

        alpha = small.tile([128, 4], F32, name="alpha", tag="alpha")
        beta = small.tile([128, 4], F32, name="beta", tag="beta")
        epsb = small.tile([128, 1], F32, name="epsb", tag="epsb")
        nc.gpsimd.memset(epsb[:], EPS)

        V = [vpool.tile([128, HW], F32R, name=f"V{dc}", tag=f"V{dc}")
             for dc in range(NCH)]
        aT = [atp.tile([128, CN], F32R, name=f"aT{dc}", tag=f"aT{dc}")
              for dc in range(NCH)]

        # ================= phase 1: proj + scores + softmax + V ==========
        with (
            tc.tile_pool(name="inp", bufs=1) as inp,
            tc.tile_pool(name="qk", bufs=2) as qkp,
            tc.tile_pool(name="attn", bufs=1) as ap_,
        ):
            qsb = inp.tile([65, HW], F32R, name="qsb", tag="qsb")
            ksb = inp.tile([65, HW], F32R, name="ksb", tag="ksb")
            vsb = inp.tile([65, HW], F32R, name="vsb", tag="vsb")
            nc.sync.dma_start(qsb[:, 0:2048], d["q"][:, 0:2048])
            nc.gpsimd.dma_start(ksb[:, 0:2048], d["k"][:, 0:2048])
            nc.sync.dma_start(qsb[:, 2048:4096], d["q"][:, 2048:4096])
            nc.gpsimd.dma_start(ksb[:, 2048:4096], d["k"][:, 2048:4096])
            nc.scalar.dma_start(vsb[:, 0:2048], d["v"][:, 0:2048])
            nc.scalar.dma_start(vsb[:, 2048:4096], d["v"][:, 2048:4096])

            rowsum = ap_.tile([128, 4], F32, name="rowsum", tag="rowsum")
            recip = ap_.tile([128, 4], F32, name="recip", tag="recip")
            attn = [ap_.tile([128, 512], F32R, name=f"attn{cc}",
                             tag=f"attn{cc}") for cc in range(NCH)]

            with (
                tc.tile_pool(name="scps", bufs=1, space="PSUM") as scps,
                tc.tile_pool(name="pjps", bufs=1, space="PSUM") as pjps,
            ):
                sc = [scps.tile([128, 512], F32, name=f"sc{cc}",
                                tag=f"sc{cc}") for cc in range(NCH)]
                # 2 m-chunks per iteration, software-pipelined one
                # iteration ahead so PE never waits on the psum->sbuf copies
                NIT = NM // 2

                def proj(it):
                    qtp = pjps.tile([128, 1024], F32, name="qtp", tag="qtp")
                    ktp = pjps.tile([128, 1024], F32, name="ktp", tag="ktp")
                    for h in range(2):
                        msl = slice(128 * (2 * it + h),
                                    128 * (2 * it + h + 1))
                        nc.tensor.matmul(qtp[:, 512 * h:512 * (h + 1)],
                                         qsb[:, msl], wqe,
                                         start=True, stop=True)
                        nc.tensor.matmul(ktp[:, 512 * h:512 * (h + 1)],
                                         ksb[:, msl], wke,
                                         start=True, stop=True)
                    return qtp, ktp

                def copies(qtp, ktp):
                    qts = qkp.tile([128, 1024], F32R, name="qts", tag="qts")
                    nc.scalar.copy(qts[:], qtp[:])
                    kts = qkp.tile([128, 1024], F32R, name="kts", tag="kts")
                    nc.vector.tensor_copy(kts[:], ktp[:])
                    return qts, kts

                def score_mms(it, qts, kts):
                    for h in range(2):
                        for cc in range(NCH):
                            nc.tensor.matmul(
                                sc[cc][:],
                                qts[:, 512 * h + 128 * cc:
                                    512 * h + 128 * (cc + 1)],
                                kts[:, 512 * h:512 * (h + 1)],
                                start=(it == 0 and h == 0),
                                stop=(it == NIT - 1 and h == 1))

                pq = proj(0)
                cur = copies(*pq)
                nxt_p = proj(1)
                for it in range(NIT):
                    score_mms(it, *cur)
                    if it + 1 < NIT:
                        cur = copies(*nxt_p)
                    if it + 2 < NIT:
                        nxt_p = proj(it + 2)

                # softmax exp on ACT (overlaps V projection below on PE)
                for cc in range(NCH):
                    nc.scalar.activation(attn[cc][:], sc[cc][:], AF.Exp,
                                         bias=0.0, scale=INV_SCALE,
                                         accum_out=rowsum[:, cc:cc + 1])

                # V projection (natural [d, m] layout)
                for dc in range(NCH):
                    for mp in range(NS // 2):
                        vtp = pjps.tile([128, 1024], F32, name="vtp",
                                        tag=("qtp" if mp % 2 == 0 else "ktp"))
                        for h in range(2):
                            ssl = slice(1024 * mp + 512 * h,
                                        1024 * mp + 512 * (h + 1))
                            nc.tensor.matmul(
                                vtp[:, 512 * h:512 * (h + 1)],
                                wve[:, 128 * dc:128 * (dc + 1)],
                                vsb[:, ssl], start=True, stop=True)
                        vsl = slice(1024 * mp, 1024 * (mp + 1))
                        if (dc + mp) % 2 == 0:
                            nc.scalar.copy(V[dc][:, vsl], vtp[:])
                        else:
                            nc.vector.tensor_copy(V[dc][:, vsl], vtp[:])

                # normalize attn rows (DVE)
                for cc in range(NCH):
                    nc.vector.reciprocal(recip[:, cc:cc + 1],
                                         rowsum[:, cc:cc + 1])
                    nc.vector.tensor_scalar_mul(attn[cc][:], attn[cc][:],
                                                recip[:, cc:cc + 1])

            # transposes: 4 blocks of one dc into one 512-wide psum tile
            with tc.tile_pool(name="tps", bufs=2, space="PSUM") as tps:
                for dc in range(NCH):
                    tp = tps.tile([128, 512], F32, name="tp", tag="tp")
                    for cc in range(NCH):
                        nc.tensor.transpose(
                            _r(tp[:, 128 * cc:128 * (cc + 1)]),
                            attn[cc][:, 128 * dc:128 * (dc + 1)],
                            ident)
                    nc.scalar.copy(aT[dc][:], tp[:])

            if dbg is not None:
                for cc in range(NCH):
                    nc.sync.dma_start(dbg[f"attn{cc}"][:], attn[cc][:])

        if dbg is not None:
            for dc in range(NCH):
                nc.sync.dma_start(dbg[f"aT{dc}"][:], aT[dc][:])
                nc.sync.dma_start(dbg[f"V{dc}"][:], V[dc][:])

        # ============ phase 2: attn@V -> X (BN layout) + stats + AR ======
        with (
            tc.tile_pool(name="xbuf", bufs=1) as xpool,
            tc.tile_pool(name="stp", bufs=2) as stp,
            tc.tile_pool(name="scr", bufs=2) as scr,
            tc.tile_pool(name="cdram", bufs=1, space="DRAM") as cdram,
        ):
            X = [xpool.tile([128, HW], F32R, name=f"X{cc}", tag=f"X{cc}")
                 for cc in range(NCH)]
            red = stp.tile([128, 8], F32, name="red", tag="red", bufs=1)
            with tc.tile_pool(name="xps", bufs=3, space="PSUM") as xps:
                for cc in range(NCH):
                    Vr = [V[dc].rearrange("d (cc t lo) -> d cc lo t",
                                          cc=4, lo=8) for dc in range(NCH)]
                    ps_sum = stp.tile([128, 4], F32, name="pssum", tag="pssum")
                    ps_sq = stp.tile([128, 4], F32, name="pssq", tag="pssq")
                    for lp in range(4):
                        xt = xps.tile([128, 1024], F32, name="xt", tag="xt")
                        for h in range(2):
                            lo = 2 * lp + h
                            for dc in range(NCH):
                                nc.tensor.matmul(
                                    xt[:, 512 * h:512 * (h + 1)],
                                    Vr[dc][:, cc, lo, :], aT[dc][:],
                                    start=(dc == 0), stop=(dc == 3))
                        xsl = slice(1024 * lp, 1024 * (lp + 1))
                        nc.vector.tensor_scalar(
                            out=X[cc][:, xsl], in0=xt[:], scalar1=1.0,
                            scalar2=0.0, op0=ALU.mult, op1=ALU.add,
                            accum_out=ps_sum[:, lp:lp + 1])
                        junk = scr.tile([128, 1024], F32, name="junk",
                                        tag="junk")
                        nc.scalar.activation(junk[:], X[cc][:, xsl],
                                             AF.Square,
                                             accum_out=ps_sq[:, lp:lp + 1])
                    nc.vector.reduce_sum(red[:, 2 * cc:2 * cc + 1],
                                         ps_sum[:], axis=AX.X)
                    nc.vector.reduce_sum(red[:, 2 * cc + 1:2 * cc + 2],
                                         ps_sq[:], axis=AX.X)

                # single AllReduce for all BN stats
                cin = cdram.tile([128, 8], F32, name="cin", tag="cin")
                cout = cdram.tile([128, 8], F32, name="cout", tag="cout")
                nc.sync.dma_start(cin[:], red[:])
                nc.gpsimd.collective_compute(
                    "AllReduce", ALU.add, replica_groups=RG,
                    ins=[cin.opt()], outs=[cout.opt()])
                ar = stp.tile([128, 8], F32, name="ar", tag="ar", bufs=1)
                nc.sync.dma_start(ar[:], cout[:])
                # preload ACT tables while the AllReduce is in flight
                dummy = stp.tile([128, 1], F32, name="dummy", tag="dummy",
                                 bufs=1)
                nc.scalar.activation(dummy[:], epsb[:], AF.Lrelu,
                                     bias=0.0, scale=1.0, alpha=SLOPE)
                nc.scalar.activation(dummy[:], epsb[:], AF.Sqrt,
                                     bias=epsb[:, 0:1])

                # BN affine params (tiny DVE/ACT ops)
                mean = stp.tile([128, 4], F32, name="mean", tag="mean",
                                bufs=1)
                var = stp.tile([128, 4], F32, name="var", tag="var", bufs=1)
                sd = stp.tile([128, 4], F32, name="sd", tag="sd", bufs=1)
                rstd = stp.tile([128, 4], F32, name="rstd", tag="rstd",
                                bufs=1)
                tmp = stp.tile([128, 4], F32, name="tmpb", tag="tmpb",
                               bufs=1)
                for cc in range(NCH):
                    nc.vector.tensor_scalar_mul(mean[:, cc:cc + 1],
                                                ar[:, 2 * cc:2 * cc + 1],
                                                INV_BHW)
                    nc.vector.tensor_scalar_mul(var[:, cc:cc + 1],
                                                ar[:, 2 * cc + 1:2 * cc + 2],
                                                INV_BHW)
                nc.vector.tensor_mul(tmp[:], mean[:], mean[:])
                nc.vector.tensor_sub(var[:], var[:], tmp[:])
                nc.scalar.activation(sd[:], var[:], AF.Sqrt,
                                     bias=epsb[:, 0:1])
                nc.vector.reciprocal(rstd[:], sd[:])
                nc.vector.tensor_mul(alpha[:], bngsb[:], rstd[:])
                nc.vector.tensor_mul(tmp[:], mean[:], alpha[:])
                nc.vector.tensor_sub(beta[:], bnbsb[:], tmp[:])

                if dbg is not None:
                    for cc in range(NCH):
                        nc.sync.dma_start(dbg[f"X{cc}"][:], X[cc][:])
                        nc.sync.dma_start(dbg[f"ar{cc}"][:, 0:1],
                                          ar[:, 2 * cc:2 * cc + 1])
                        nc.sync.dma_start(dbg[f"ar{cc}"][:, 1:2],
                                          ar[:, 2 * cc + 1:2 * cc + 2])
                        nc.sync.dma_start(dbg[f"ab{cc}"][:, 0:1],
                                          alpha[:, cc:cc + 1])
                        nc.sync.dma_start(dbg[f"ab{cc}"][:, 1:2],
                                          beta[:, cc:cc + 1])

                # BN + leaky, in place; first 512 cols of each chunk
                # first so the w1 phase unblocks early
                for xsl in (slice(0, 512), slice(512, 2048),
                            slice(2048, 4096)):
                    for cc in range(NCH):
                        nc.scalar.activation(X[cc][:, xsl], X[cc][:, xsl],
                                             AF.Lrelu,
                                             bias=beta[:, cc:cc + 1],
                                             scale=alpha[:, cc:cc + 1],
                                             alpha=SLOPE)

            # ================= phase 3: w1 -> leaky -> w2 -> y ===========
            with (
                tc.tile_pool(name="y2", bufs=2) as y2p,
                tc.tile_pool(name="outb", bufs=1) as outp,
                tc.tile_pool(name="wps", bufs=1, space="PSUM") as wps,
                tc.tile_pool(name="w2ps", bufs=2, space="PSUM") as w2ps,
            ):
                osb = outp.tile([64, HW], F32, name="osb", tag="osb")
                for ms in range(NS):
                    ssl = slice(512 * ms, 512 * (ms + 1))
                    y2t = []
                    for oc in range(NCH):
                        wp = wps.tile([128, 512], F32, name=f"wp{oc}",
                                      tag=f"wp{oc}")
                        for cc in range(NCH):
                            nc.tensor.matmul(
                                wp[:],
                                w1sb[cc][:, 128 * oc:128 * (oc + 1)],
                                X[cc][:, ssl],
                                start=(cc == 0), stop=(cc == 3))
                        yt = y2p.tile([128, 512], F32R, name=f"y2_{oc}",
                                      tag=f"y2_{oc}")
                        nc.scalar.activation(yt[:], wp[:], AF.Lrelu,
                                             bias=b1sb[:, oc:oc + 1],
                                             scale=1.0, alpha=SLOPE)
                        y2t.append(yt)
                    fp = w2ps.tile([64, 512], F32, name="fp", tag="fp")
                    for oc in range(NCH):
                        nc.tensor.matmul(fp[:], w2sb[oc], y2t[oc][:],
                                         start=(oc == 0), stop=(oc == 3))
                    nc.vector.tensor_scalar_add(osb[:, ssl], fp[:],
                                                b2sb[:, 0:1])
                    nc.sync.dma_start(d["y"][:, ssl], osb[:, ssl])


_NC_CACHE = {}


def _build(debug=False):
    key = ("dbg" if debug else "nc")
    if key in _NC_CACHE:
        return _NC_CACHE[key]
    nc = bacc.Bacc(trn_type="TRN2", target_bir_lowering=False, debug=False,
                   enable_asserts=False, num_devices=8)
    d = {}
    d["q"] = nc.dram_tensor("q", (65, HW), F32R, kind="ExternalInput").ap()
    d["k"] = nc.dram_tensor("k", (65, HW), F32R, kind="ExternalInput").ap()
    d["v"] = nc.dram_tensor("v", (65, HW), F32R, kind="ExternalInput").ap()
    d["wpack"] = nc.dram_tensor("wpack", (128, 4000), F32R,
                                kind="ExternalInput").ap()
    d["y"] = nc.dram_tensor("y", (64, HW), F32, kind="ExternalOutput").ap()

    dbg = None
    if debug:
        dbg = {}
        for cc in range(NCH):
            dbg[f"attn{cc}"] = nc.dram_tensor(f"dbg_attn{cc}", (128, 512), F32R, kind="ExternalOutput").ap()
            dbg[f"aT{cc}"] = nc.dram_tensor(f"dbg_aT{cc}", (128, 512), F32R, kind="ExternalOutput").ap()
            dbg[f"V{cc}"] = nc.dram_tensor(f"dbg_V{cc}", (128, HW), F32R, kind="ExternalOutput").ap()
            dbg[f"X{cc}"] = nc.dram_tensor(f"dbg_X{cc}", (128, HW), F32R, kind="ExternalOutput").ap()
            dbg[f"ar{cc}"] = nc.dram_tensor(f"dbg_ar{cc}", (128, 2), F32, kind="ExternalOutput").ap()
            dbg[f"ab{cc}"] = nc.dram_tensor(f"dbg_ab{cc}", (128, 2), F32, kind="ExternalOutput").ap()
    with tile.TileContext(nc) as tc:
        _body(tc, nc, d, dbg)
    nc.compile()
    _NC_CACHE[key] = nc
    return nc


def _run(q, k, v, wq, bq, wk, bk, wv, bv, bn_g, bn_b, w1, b1, w2, b2,
         trace=False, tmpdir=None, debug=False):
    nc = _build(debug)
    f = np.float32
    wpack = np.zeros((128, 4000), f)
    wpack[0:65, 0:512] = np.concatenate([wq.T, bq[None, :]], axis=0)
    wpack[0:65, 512:1024] = np.concatenate([wk.T, bk[None, :]], axis=0)
    wpack[0:65, 1024:1536] = np.concatenate([wv.T, bv[None, :]], axis=0)
    w1t = w1.T.astype(f)
    for cc in range(4):
        wpack[:, 1536 + 512 * cc:1536 + 512 * (cc + 1)] = \
            w1t[128 * cc:128 * (cc + 1), :]
    w2t = w2.T.astype(f)
    for oc in range(4):
        wpack[:, 3584 + 64 * oc:3584 + 64 * (oc + 1)] = \
            w2t[128 * oc:128 * (oc + 1), :]
    wpack[:, 3840:3968] = np.eye(128, dtype=f)
    wpack[:, 3968:3972] = b1.reshape(4, 128).T
    wpack[0:64, 3972] = b2
    wpack[:, 3976:3980] = bn_g.reshape(4, 128).T
    wpack[:, 3980:3984] = bn_b.reshape(4, 128).T
    shared = {"wpack": wpack}
    in_maps = []
    for b in range(B):
        m = dict(shared)
        ones = np.ones((1, HW), f)
        m["q"] = np.concatenate([q[b].reshape(64, HW), ones], axis=0).astype(f)
        m["k"] = np.concatenate([k[b].reshape(64, HW), ones], axis=0).astype(f)
        m["v"] = np.concatenate([v[b].reshape(64, HW), ones], axis=0).astype(f)
        in_maps.append(m)
    res = bass_utils.run_bass_kernel_spmd(
        nc, in_maps, core_ids=list(range(8)), trace=trace, tmpdir=tmpdir)
    out = np.stack([res.results[b]["y"].reshape(C, 64, 64) for b in range(B)])
    return out.astype(np.float32), res


def kernel(q, k, v, wq, bq, wk, bk, wv, bv, bn_g, bn_b, w1, b1, w2, b2):
    out, _ = _run(q, k, v, wq, bq, wk, bk, wv, bv, bn_g, bn_b, w1, b1, w2, b2)
    return out



# revision 22
# speedup vs baseline: 1.4637x; 1.4637x over previous
"""Trainium2 Bass kernel: channel-attention MultiHeadAttention block.

Full (unsharded) inputs in, full output out. Data-parallel over batch B
across 8 NeuronCores (1 batch each) with one tiny AllReduce for BatchNorm
batch statistics.

Algebraic structure (per core, batch b). All projections are folded through
the 65x65 Gram matrix so the PE never materializes Q/K/V:

  qe/ke/ve  [65, 4096]   raw inputs + ones row (bias fold), bf16
  G         [65, 65]     = qe @ ke^T     (32 small accumulating matmuls on
                           host-pre-transposed qT/kT chunks)
  scores    [512, 512]   = wqe^T G wke   (tiny expansion: GT, H=G@wke, 4 mm)
  attn      [128c,512d]  exp(scores/64)/rowsum (ACT exp + DVE normalize)
  aT        [128d,512c]  XBAR DMA-transpose of attn (no PE)
  A2        [65, 512]    = wv_ext^T @ attn^T  (4 matmuls; wv_ext=[wv|bv])
  X[cc]     [128c',4096] = A2^T @ ve via stride-8 ve slices as stationary:
                           X[c', 512r+c] = attnout[c, 8c'+r]  (BN layout)
  BN stats  copy+sum (ACT) / square+sum (DVE) -> AllReduce -> alpha/beta
  BN+leaky  in-place ACT Lrelu(scale=alpha, bias=beta)
  w1+leaky  [512,512] conv (bf16), bias+leaky fused in ACT
  w2+bias   [64,512] conv -> y [64, 4096] f32
"""

import sys

if "/opt/trn_rl_repo" not in sys.path:
    sys.path.insert(0, "/opt/trn_rl_repo")

import numpy as np

import concourse.bacc as bacc
import concourse.mybir as mybir
import concourse.tile as tile
from concourse import bass_utils

B = 8
C = 64
CN = 512
HW = 4096
NCH = CN // 128  # 4 channel chunks
NS = HW // 512   # 8 free-dim slices
NMC = HW // 128  # 32 m-chunks for the Gram accumulation
EPS = 1e-4
SLOPE = 0.01
INV_SCALE = 1.0 / 64.0      # 1/sqrt(HW)
INV_BHW = 1.0 / (B * HW)    # BN divisor

F32 = mybir.dt.float32
F32R = mybir.dt.float32r
BF16 = mybir.dt.bfloat16
AF = mybir.ActivationFunctionType
ALU = mybir.AluOpType
AX = mybir.AxisListType
RG = [[0, 1, 2, 3, 4, 5, 6, 7]]

# wpk32 f32 column layout
W32_WQE = 0
W32_WKE = 512
W32_ID = 1024      # eye(128) at [:, 1024:1152]
W32_B1 = 1152      # [128, 4]
W32_B2 = 1156      # [64, 1]
W32_BNG = 1160     # [128, 4]
W32_BNB = 1164     # [128, 4]
W32_N = 1168

# wpk16 bf16 column layout
W16_W1 = 0         # 4 x [128, 512]
W16_W2 = 2048      # 4 x [128, 64]
W16_WVT = 2304     # 4 x [128, 65]
W16_N = 2564


def _body(tc, nc, d, dbg=None):
    with (
        tc.tile_pool(name="consts", bufs=1) as consts,
        tc.tile_pool(name="small", bufs=1) as small,
    ):
        # ---- inputs + weights (DMA queues: sync=SP, scalar=ACT, gpsimd)
        qt = consts.tile([128, 32 * 65], BF16, name="qt", tag="qt")
        kt = consts.tile([128, 32 * 65], BF16, name="kt", tag="kt")
        vsb = consts.tile([65, HW], BF16, name="vsb", tag="vsb")
        wpk32 = consts.tile([128, W32_N], F32R, name="wpk32", tag="wpk32")
        wpk16 = consts.tile([128, W16_N], BF16, name="wpk16", tag="wpk16")
        nc.sync.dma_start(qt[:], d["qt"][:])
        nc.scalar.dma_start(kt[:], d["kt"][:])
        nc.sync.dma_start(wpk32[:], d["wpk32"][:])
        nc.scalar.dma_start(wpk16[:], d["wpk16"][:])
        nc.scalar.dma_start(vsb[:], d["v"][:])

        wqe = wpk32[0:65, W32_WQE:W32_WQE + 512]
        wke = wpk32[0:65, W32_WKE:W32_WKE + 512]
        ident = wpk32[:, W32_ID:W32_ID + 128]
        b1sb = wpk32[:, W32_B1:W32_B1 + 4].bitcast(F32)
        b2sb = wpk32[0:64, W32_B2:W32_B2 + 1].bitcast(F32)
        bngsb = wpk32[:, W32_BNG:W32_BNG + 4].bitcast(F32)
        bnbsb = wpk32[:, W32_BNB:W32_BNB + 4].bitcast(F32)
        w1sb = [wpk16[:, W16_W1 + 512 * cc:W16_W1 + 512 * (cc + 1)]
                for cc in range(NCH)]
        w2sb = [wpk16[:, W16_W2 + 64 * oc:W16_W2 + 64 * (oc + 1)]
                for oc in range(NCH)]
        wvt = [wpk16[:, W16_WVT + 65 * j:W16_WVT + 65 * (j + 1)]
               for j in range(NCH)]

        # warmup collective: absorbs ncfw cold-start + inter-core skew early
        with tc.tile_pool(name="wudram", bufs=1, space="DRAM") as wud:
            dwin = wud.tile([128, 1], F32, name="dwin", tag="dwin")
            dwout = wud.tile([128, 1], F32, name="dwout", tag="dwout")
            wsrc = small.tile([128, 1], F32, name="wsrc", tag="wsrc")
            nc.gpsimd.memset(wsrc[:], 1.0)
            nc.gpsimd.dma_start(dwin[:], wsrc[:])
            nc.gpsimd.collective_compute(
                "AllReduce", ALU.add, replica_groups=RG,
                ins=[dwin.opt()], outs=[dwout.opt()])
            wdst = small.tile([128, 1], F32, name="wdst", tag="wdst")
            nc.gpsimd.dma_start(wdst[:], dwout[:])

        # ACT Exp table preload while DMAs are in flight (one table resident
        # at a time; later sets are loaded behind other work, see below)
        epsb = small.tile([128, 1], F32, name="epsb", tag="epsb")
        nc.vector.memset(epsb[:], EPS)
        dummy = small.tile([128, 1], F32, name="dummy", tag="dummy")
        nc.scalar.activation(dummy[:], epsb[:], AF.Exp, bias=0.0, scale=1.0)

        alpha = small.tile([128, 4], F32, name="alpha", tag="alpha")
        beta = small.tile([128, 4], F32, name="beta", tag="beta")

        X = [consts.tile([128, HW], BF16, name=f"X{cc}", tag=f"X{cc}")
             for cc in range(NCH)]
        aTall = consts.tile([128, CN * NCH], BF16, name="aTall", tag="aTall")
        A2sb = consts.tile([65, CN], BF16, name="A2sb", tag="A2sb")

        # ================= phase 1: Gram -> scores -> attn -> A2 =========
        with tc.tile_pool(name="sc32", bufs=1) as sc32:
            with tc.tile_pool(name="ps_g", bufs=1, space="PSUM") as psg:
                # GT = ke @ qe^T [65cj, 65ci] (Gram, swapped: no transpose)
                GTps = psg.tile([65, 65], F32, name="GTps", tag="GTps")
                for j in range(NMC):
                    nc.tensor.matmul(GTps[:], kt[:, 65 * j:65 * (j + 1)],
                                     qt[:, 65 * j:65 * (j + 1)],
                                     start=(j == 0), stop=(j == NMC - 1))
                GTsb = sc32.tile([65, 65], F32R, name="GTsb", tag="GTsb")
                nc.vector.tensor_copy(GTsb[:], GTps[:])
                # H = G @ wke = GT^T @ wke  [65ci, 512]
                Hps = psg.tile([65, 512], F32, name="Hps", tag="Hps")
                nc.tensor.matmul(Hps[:], GTsb[:], wke, start=True, stop=True)
                Hsb = sc32.tile([65, 512], F32R, name="Hsb", tag="Hsb")
                nc.scalar.copy(Hsb[:], Hps[:])

            # scores chunks + exp + normalize
            attnb = sc32.tile([128, CN * NCH], F32R, name="attnb",
                              tag="attnb")
            rowsum = small.tile([128, 4], F32, name="rowsum", tag="rowsum")
            recip = small.tile([128, 4], F32, name="recip", tag="recip")
            with tc.tile_pool(name="ps_sc", bufs=1, space="PSUM") as pssc:
                sc = [pssc.tile([128, 512], F32, name=f"sc{cc}",
                                tag=f"sc{cc}") for cc in range(NCH)]
                for cc in range(NCH):
                    nc.tensor.matmul(sc[cc][:],
                                     wqe[:, 128 * cc:128 * (cc + 1)],
                                     Hsb[:], start=True, stop=True)
                    nc.scalar.activation(attnb[:, 512 * cc:512 * (cc + 1)],
                                         sc[cc][:], AF.Exp,
                                         bias=0.0, scale=INV_SCALE,
                                         accum_out=rowsum[:, cc:cc + 1])
                # Sqrt table load hidden behind X2/stats work (needed at sd)
                nc.scalar.activation(dummy[:], epsb[:], AF.Sqrt,
                                     bias=epsb[:, 0:1])
                nc.vector.reciprocal(recip[:], rowsum[:])
                for cc in range(NCH):
                    nc.vector.tensor_scalar_mul(
                        attnb[:, 512 * cc:512 * (cc + 1)],
                        attnb[:, 512 * cc:512 * (cc + 1)],
                        recip[:, cc:cc + 1])

            # aT via PE transposes: block j holds d in [128j, 128j+128)
            with (
                tc.tile_pool(name="tps", bufs=2, space="PSUM") as tps,
                tc.tile_pool(name="ps_a2", bufs=1, space="PSUM") as psa2,
            ):
                for j in range(NCH):
                    tp = tps.tile([128, 512], F32R, name="tp", tag="tp")
                    for cc in range(NCH):
                        nc.tensor.transpose(
                            tp[:, 128 * cc:128 * (cc + 1)],
                            attnb[:, 512 * cc + 128 * j:
                                   512 * cc + 128 * (j + 1)],
                            ident)
                    if j % 2 == 0:
                        nc.vector.tensor_copy(
                            aTall[:, 512 * j:512 * (j + 1)], tp[:])
                    else:
                        nc.scalar.copy(
                            aTall[:, 512 * j:512 * (j + 1)], tp[:])

                # A2 = wv_ext^T @ attn^T  [65, 512]
                A2ps = psa2.tile([65, 512], F32, name="A2ps", tag="A2ps")
                for j in range(NCH):
                    nc.tensor.matmul(A2ps[:], wvt[j],
                                     aTall[:, 512 * j:512 * (j + 1)],
                                     start=(j == 0), stop=(j == NCH - 1))
                nc.scalar.copy(A2sb[:], A2ps[:])

            if dbg is not None:
                nc.sync.dma_start(dbg["attnb"][:], attnb[:])

        if dbg is not None:
            nc.sync.dma_start(dbg["aTall"][:], aTall[:])
            nc.sync.dma_start(dbg["A2sb"][:], A2sb[:])

        # ============ phase 2: X (BN layout) + stats + AllReduce =========
        vre = vsb[:, :].rearrange("ci (cp r) -> ci r cp", r=8)
        with (
            tc.tile_pool(name="stp", bufs=1) as stp,
            tc.tile_pool(name="junkp", bufs=2) as junkp,
            tc.tile_pool(name="cdram", bufs=1, space="DRAM") as cdram,
            tc.tile_pool(name="ps_x", bufs=3, space="PSUM") as psx,
        ):
            pssum = stp.tile([128, 16], F32, name="pssum", tag="pssum")
            pssq = stp.tile([128, 16], F32, name="pssq", tag="pssq")
            red = stp.tile([128, 8], F32, name="red", tag="red")
            for cc in range(NCH):
                for rp in range(4):
                    xt = psx.tile([128, 1024], F32, name="xt", tag="xt")
                    for h in range(2):
                        r = 2 * rp + h
                        nc.tensor.matmul(
                            xt[:, 512 * h:512 * (h + 1)],
                            vre[:, r, 128 * cc:128 * (cc + 1)],
                            A2sb[:], start=True, stop=True)
                    slot = 4 * cc + rp
                    xsl = slice(1024 * rp, 1024 * (rp + 1))
                    nc.scalar.activation(X[cc][:, xsl], xt[:], AF.Copy,
                                         accum_out=pssum[:, slot:slot + 1])
                    junk = junkp.tile([128, 1024], BF16, name="junk",
                                      tag="junk")
                    nc.vector.scalar_tensor_tensor(
                        junk[:], X[cc][:, xsl], 1.0, X[cc][:, xsl],
                        op0=ALU.bypass, op1=ALU.mult,
                        accum_out=pssq[:, slot:slot + 1])
            for cc in range(NCH):
                nc.vector.reduce_sum(red[:, 2 * cc:2 * cc + 1],
                                     pssum[:, 4 * cc:4 * (cc + 1)], axis=AX.X)
                nc.vector.reduce_sum(red[:, 2 * cc + 1:2 * cc + 2],
                                     pssq[:, 4 * cc:4 * (cc + 1)], axis=AX.X)

            # single AllReduce for all BN stats
            cin = cdram.tile([128, 8], F32, name="cin", tag="cin")
            cout = cdram.tile([128, 8], F32, name="cout", tag="cout")
            nc.sync.dma_start(cin[:], red[:])
            nc.gpsimd.collective_compute(
                "AllReduce", ALU.add, replica_groups=RG,
                ins=[cin.opt()], outs=[cout.opt()])
            ar = stp.tile([128, 8], F32, name="ar", tag="ar")
            nc.sync.dma_start(ar[:], cout[:])

            # BN affine params
            mean = stp.tile([128, 4], F32, name="mean", tag="mean")
            var = stp.tile([128, 4], F32, name="var", tag="var")
            sd = stp.tile([128, 4], F32, name="sd", tag="sd")
            rstd = stp.tile([128, 4], F32, name="rstd", tag="rstd")
            tmp = stp.tile([128, 4], F32, name="tmpb", tag="tmpb")
            nc.vector.tensor_scalar_mul(
                mean[:], ar[:, :].rearrange("p (c two) -> p two c", two=2)[:, 0, :],
                INV_BHW)
            nc.vector.tensor_scalar_mul(
                var[:], ar[:, :].rearrange("p (c two) -> p two c", two=2)[:, 1, :],
                INV_BHW)
            nc.vector.tensor_mul(tmp[:], mean[:], mean[:])
            nc.vector.tensor_sub(var[:], var[:], tmp[:])
            nc.scalar.activation(sd[:], var[:], AF.Sqrt, bias=epsb[:, 0:1])
            # Lrelu table load overlaps the DVE alpha/beta chain below
            nc.scalar.activation(dummy[:], epsb[:], AF.Lrelu,
                                 bias=0.0, scale=1.0, alpha=SLOPE)
            nc.vector.reciprocal(rstd[:], sd[:])
            nc.vector.tensor_mul(alpha[:], bngsb[:], rstd[:])
            nc.vector.tensor_mul(tmp[:], mean[:], alpha[:])
            nc.vector.tensor_sub(beta[:], bnbsb[:], tmp[:])
            if dbg is not None:
                # experiment: rstd via DVE pow ALU (var+eps)^-0.5
                powt = stp.tile([128, 4], F32, name="powt", tag="powt")
                nc.vector.tensor_scalar(
                    out=powt[:], in0=var[:], scalar1=EPS, scalar2=-0.5,
                    op0=ALU.add, op1=ALU.pow)
                nc.sync.dma_start(dbg["powt"][:], powt[:])

            if dbg is not None:
                for cc in range(NCH):
                    nc.sync.dma_start(dbg[f"X{cc}"][:], X[cc][:])
                nc.sync.dma_start(dbg["red"][:], red[:])
                nc.sync.dma_start(dbg["ar"][:], ar[:])
                nc.sync.dma_start(dbg["ab"][:, 0:4], alpha[:])
                nc.sync.dma_start(dbg["ab"][:, 4:8], beta[:])

            # BN + leaky in place, ms-pair-major so w1 unblocks early
            for mp in range(4):
                for cc in range(NCH):
                    xsl = slice(1024 * mp, 1024 * (mp + 1))
                    nc.scalar.activation(X[cc][:, xsl], X[cc][:, xsl],
                                         AF.Lrelu, bias=beta[:, cc:cc + 1],
                                         scale=alpha[:, cc:cc + 1],
                                         alpha=SLOPE)

        # ================= phase 3: w1 -> leaky -> w2 -> y ===============
        with (
            tc.tile_pool(name="y2", bufs=2) as y2p,
            tc.tile_pool(name="outb", bufs=1) as outp,
            tc.tile_pool(name="wps", bufs=4, space="PSUM") as wps,
            tc.tile_pool(name="w2ps", bufs=2, space="PSUM") as w2ps,
        ):
            osb = outp.tile([64, HW], F32, name="osb", tag="osb")
            for ms in range(NS):
                ssl = slice(512 * ms, 512 * (ms + 1))
                y2t = []
                for oc in range(NCH):
                    wp = wps.tile([128, 512], F32, name="wp", tag="wp")
                    for cc in range(NCH):
                        nc.tensor.matmul(
                            wp[:], w1sb[cc][:, 128 * oc:128 * (oc + 1)],
                            X[cc][:, ssl], start=(cc == 0), stop=(cc == 3))
                    yt = y2p.tile([128, 512], BF16, name=f"y2_{oc}",
                                  tag=f"y2_{oc}")
                    nc.scalar.activation(yt[:], wp[:], AF.Lrelu,
                                         bias=b1sb[:, oc:oc + 1],
                                         scale=1.0, alpha=SLOPE)
                    y2t.append(yt)
                fp = w2ps.tile([64, 512], F32, name="fp", tag="fp")
                for oc in range(NCH):
                    nc.tensor.matmul(fp[:], w2sb[oc], y2t[oc][:],
                                     start=(oc == 0), stop=(oc == 3))
                nc.vector.tensor_scalar_add(osb[:, ssl], fp[:],
                                            b2sb[:, 0:1])
                nc.sync.dma_start(d["y"][:, ssl], osb[:, ssl])


_NC_CACHE = {}


def _build(debug=False):
    key = ("dbg" if debug else "nc")
    if key in _NC_CACHE:
        return _NC_CACHE[key]
    nc = bacc.Bacc(trn_type="TRN2", target_bir_lowering=False, debug=False,
                   enable_asserts=False, num_devices=8)
    d = {}
    d["qt"] = nc.dram_tensor("qt", (128, 32 * 65), BF16,
                             kind="ExternalInput").ap()
    d["kt"] = nc.dram_tensor("kt", (128, 32 * 65), BF16,
                             kind="ExternalInput").ap()
    d["v"] = nc.dram_tensor("v", (65, HW), BF16, kind="ExternalInput").ap()
    d["wpk32"] = nc.dram_tensor("wpk32", (128, W32_N), F32R,
                                kind="ExternalInput").ap()
    d["wpk16"] = nc.dram_tensor("wpk16", (128, W16_N), BF16,
                                kind="ExternalInput").ap()
    d["y"] = nc.dram_tensor("y", (64, HW), F32, kind="ExternalOutput").ap()

    dbg = None
    if debug:
        dbg = {}
        dbg["attnb"] = nc.dram_tensor("dbg_attnb", (128, 2048), F32,
                                      kind="ExternalOutput").ap()
        dbg["powt"] = nc.dram_tensor("dbg_powt", (128, 4), F32,
                                     kind="ExternalOutput").ap()
        dbg["aTall"] = nc.dram_tensor("dbg_aTall", (128, 2048), BF16,
                                      kind="ExternalOutput").ap()
        dbg["A2sb"] = nc.dram_tensor("dbg_A2sb", (65, 512), BF16,
                                     kind="ExternalOutput").ap()
        for cc in range(NCH):
            dbg[f"X{cc}"] = nc.dram_tensor(f"dbg_X{cc}", (128, HW), BF16,
                                           kind="ExternalOutput").ap()
        dbg["red"] = nc.dram_tensor("dbg_red", (128, 8), F32,
                                    kind="ExternalOutput").ap()
        dbg["ar"] = nc.dram_tensor("dbg_ar", (128, 8), F32,
                                   kind="ExternalOutput").ap()
        dbg["ab"] = nc.dram_tensor("dbg_ab", (128, 8), F32,
                                   kind="ExternalOutput").ap()
    with tile.TileContext(nc) as tc:
        _body(tc, nc, d, dbg)
    nc.compile()
    _NC_CACHE[key] = nc
    return nc


def _to_bf16(a):
    import ml_dtypes
    return a.astype(ml_dtypes.bfloat16)


def _run(q, k, v, wq, bq, wk, bk, wv, bv, bn_g, bn_b, w1, b1, w2, b2,
         trace=False, tmpdir=None, debug=False):
    nc = _build(debug)
    f = np.float32
    ones = np.ones((1, HW), f)

    wpk32 = np.zeros((128, W32_N), f)
    wpk32[0:65, W32_WQE:W32_WQE + 512] = np.concatenate(
        [wq.T, bq[None, :]], axis=0)
    wpk32[0:65, W32_WKE:W32_WKE + 512] = np.concatenate(
        [wk.T, bk[None, :]], axis=0)
    wpk32[:, W32_ID:W32_ID + 128] = np.eye(128, dtype=f)
    wpk32[:, W32_B1:W32_B1 + 4] = b1.reshape(4, 128).T
    wpk32[0:64, W32_B2] = b2
    wpk32[:, W32_BNG:W32_BNG + 4] = bn_g.reshape(4, 128).T
    wpk32[:, W32_BNB:W32_BNB + 4] = bn_b.reshape(4, 128).T

    wpk16 = np.zeros((128, W16_N), f)
    w1t = w1.T.astype(f)
    for cc in range(4):
        wpk16[:, W16_W1 + 512 * cc:W16_W1 + 512 * (cc + 1)] = \
            w1t[128 * cc:128 * (cc + 1), :]
    w2t = w2.T.astype(f)
    for oc in range(4):
        wpk16[:, W16_W2 + 64 * oc:W16_W2 + 64 * (oc + 1)] = \
            w2t[128 * oc:128 * (oc + 1), :]
    wv_ext = np.concatenate([wv, bv[:, None]], axis=1)  # [512, 65]
    for j in range(4):
        wpk16[:, W16_WVT + 65 * j:W16_WVT + 65 * (j + 1)] = \
            wv_ext[128 * j:128 * (j + 1), :]
    wpk16 = _to_bf16(wpk16)

    shared = {"wpk32": wpk32, "wpk16": wpk16}
    in_maps = []
    for b in range(B):
        m = dict(shared)
        for key, src in (("qt", q), ("kt", k)):
            ext = np.concatenate([src[b].reshape(64, HW), ones], axis=0)
            # pre-tiled transpose: [p, 65*j+ci] = ext[ci, 128j+p]
            m[key] = _to_bf16(
                ext.T.reshape(32, 128, 65).transpose(1, 0, 2).reshape(128, -1))
        m["v"] = _to_bf16(np.concatenate([v[b].reshape(64, HW), ones], axis=0))
        in_maps.append(m)
    res = bass_utils.run_bass_kernel_spmd(
        nc, in_maps, core_ids=list(range(8)), trace=trace, tmpdir=tmpdir)
    out = np.stack([res.results[b]["y"].reshape(C, 64, 64) for b in range(B)])
    return out.astype(np.float32), res


def kernel(q, k, v, wq, bq, wk, bk, wv, bv, bn_g, bn_b, w1, b1, w2, b2):
    out, _ = _run(q, k, v, wq, bq, wk, bk, wv, bv, bn_g, bn_b, w1, b1, w2, b2)
    return out


# revision 33
# speedup vs baseline: 1.5778x; 1.0780x over previous
"""Trainium2 Bass kernel: channel-attention MultiHeadAttention block.

Full (unsharded) inputs in, full output out. Data-parallel over batch B
across 8 NeuronCores (1 batch each) with one tiny AllReduce for BatchNorm
batch statistics.

Algebraic structure (per core, batch b). All projections are folded through
the 65x65 Gram matrix so the PE never materializes Q/K/V:

  qe/ke/ve  [65, 4096]   raw inputs + ones row (bias fold), bf16
  G         [65, 65]     = qe @ ke^T     (32 small accumulating matmuls on
                           host-pre-transposed qT/kT chunks)
  scores    [512, 512]   = wqe^T G wke   (tiny expansion: GT, H=G@wke, 4 mm)
  attn      [128c,512d]  exp(scores/64)/rowsum (ACT exp + DVE normalize)
  aT        [128d,512c]  XBAR DMA-transpose of attn (no PE)
  A2        [65, 512]    = wv_ext^T @ attn^T  (4 matmuls; wv_ext=[wv|bv])
  X[cc]     [128c',4096] = A2^T @ ve via stride-8 ve slices as stationary:
                           X[c', 512r+c] = attnout[c, 8c'+r]  (BN layout)
  BN stats  copy+sum (ACT) / square+sum (DVE) -> AllReduce -> alpha/beta
  BN+leaky  in-place ACT Lrelu(scale=alpha, bias=beta)
  w1+leaky  [512,512] conv (bf16), bias+leaky fused in ACT
  w2+bias   [64,512] conv -> y [64, 4096] f32
"""

import sys

if "/opt/trn_rl_repo" not in sys.path:
    sys.path.insert(0, "/opt/trn_rl_repo")

import numpy as np

import concourse.bacc as bacc
import concourse.mybir as mybir
import concourse.tile as tile
from concourse import bass_utils

B = 8
C = 64
CN = 512
HW = 4096
NCH = CN // 128  # 4 channel chunks
NS = HW // 512   # 8 free-dim slices
NMC = HW // 128  # 32 m-chunks for the Gram accumulation
EPS = 1e-4
SLOPE = 0.01
INV_SCALE = 1.0 / 64.0      # 1/sqrt(HW)
INV_BHW = 1.0 / (B * HW)    # BN divisor

F32 = mybir.dt.float32
F32R = mybir.dt.float32r
BF16 = mybir.dt.bfloat16
AF = mybir.ActivationFunctionType
ALU = mybir.AluOpType
AX = mybir.AxisListType
RG = [[0, 1, 2, 3, 4, 5, 6, 7]]

# wmisc f32 column layout
WM_ID = 0          # eye(128)
WM_B1 = 128        # [128, 4]
WM_B2 = 132        # [64, 1]
WM_BNG = 136       # [128, 4]
WM_BNB = 140       # [128, 4]
WM_N = 144

# wpk16 bf16 column layout
W16_W1 = 0         # 4 x [128, 512]
W16_W2 = 2048      # 4 x [128, 64]
W16_WVT = 2304     # 4 x [128, 65]
W16_N = 2564


def _body(tc, nc, d, dbg=None):
    with (
        tc.tile_pool(name="consts", bufs=1) as consts,
        tc.tile_pool(name="small", bufs=1) as small,
    ):
        # ---- inputs + weights. qt/kt split into quarters round-robined
        # over the 3 DMA queues (sync=SP, scalar=ACT, gpsimd) so the Gram
        # matmuls can start as soon as the first chunks land.
        qt = consts.tile([128, 32 * 65], BF16, name="qt", tag="qt")
        kt = consts.tile([128, 32 * 65], BF16, name="kt", tag="kt")
        vsb = consts.tile([65, HW], BF16, name="vsb", tag="vsb")
        wqek = consts.tile([65, 1024], F32R, name="wqek", tag="wqek")
        wmisc = consts.tile([128, WM_N], F32R, name="wmisc", tag="wmisc")
        wpk16 = consts.tile([128, W16_N], BF16, name="wpk16", tag="wpk16")
        QD = 520  # 8 Gram chunks per quarter
        for g in range(4):
            qsl = slice(QD * g, QD * (g + 1))
            qeng = (nc.sync, nc.gpsimd, nc.scalar, nc.sync)[g]
            keng = (nc.scalar, nc.sync, nc.gpsimd, nc.scalar)[g]
            qeng.dma_start(qt[:, qsl], d["qt"][:, qsl])
            keng.dma_start(kt[:, qsl], d["kt"][:, qsl])
        nc.gpsimd.dma_start(wqek[:], d["wqek"][:])
        nc.gpsimd.dma_start(wmisc[:], d["wmisc"][:])
        nc.scalar.dma_start(wpk16[:, W16_WVT:W16_N],
                            d["wpk16"][:, W16_WVT:W16_N])
        nc.gpsimd.dma_start(vsb[:], d["v"][:])
        nc.sync.dma_start(wpk16[:, 0:W16_WVT], d["wpk16"][:, 0:W16_WVT])

        wqe = wqek[0:65, 0:512]
        wke = wqek[0:65, 512:1024]
        ident = wmisc[:, WM_ID:WM_ID + 128]
        b1sb = wmisc[:, WM_B1:WM_B1 + 4].bitcast(F32)
        b2sb = wmisc[0:64, WM_B2:WM_B2 + 1].bitcast(F32)
        bngsb = wmisc[:, WM_BNG:WM_BNG + 4].bitcast(F32)
        bnbsb = wmisc[:, WM_BNB:WM_BNB + 4].bitcast(F32)
        w1sb = [wpk16[:, W16_W1 + 512 * cc:W16_W1 + 512 * (cc + 1)]
                for cc in range(NCH)]
        w2sb = [wpk16[:, W16_W2 + 64 * oc:W16_W2 + 64 * (oc + 1)]
                for oc in range(NCH)]
        wvt = [wpk16[:, W16_WVT + 65 * j:W16_WVT + 65 * (j + 1)]
               for j in range(NCH)]

        # ACT Exp table preload while DMAs are in flight (one table resident
        # at a time; the Lrelu set is loaded behind the stats phase below)
        epsb = small.tile([128, 1], F32, name="epsb", tag="epsb")
        nc.vector.memset(epsb[:], EPS)
        dummy = small.tile([128, 1], F32, name="dummy", tag="dummy")
        nc.scalar.activation(dummy[:], epsb[:], AF.Exp, bias=0.0, scale=1.0)

        alpha = small.tile([128, 4], F32, name="alpha", tag="alpha")
        beta = small.tile([128, 4], F32, name="beta", tag="beta")

        X = [consts.tile([128, HW], BF16, name=f"X{cc}", tag=f"X{cc}")
             for cc in range(NCH)]
        aTall = consts.tile([128, CN * NCH], BF16, name="aTall", tag="aTall")
        A2sb = consts.tile([65, CN], BF16, name="A2sb", tag="A2sb")

        # ================= phase 1: Gram -> scores -> attn -> A2 =========
        with tc.tile_pool(name="sc32", bufs=1) as sc32:
            with tc.tile_pool(name="ps_g", bufs=1, space="PSUM") as psg:
                # GT = ke @ qe^T [65cj, 65ci] (Gram, swapped: no transpose)
                GTps = psg.tile([65, 65], F32, name="GTps", tag="GTps")
                for j in range(NMC):
                    nc.tensor.matmul(GTps[:], kt[:, 65 * j:65 * (j + 1)],
                                     qt[:, 65 * j:65 * (j + 1)],
                                     start=(j == 0), stop=(j == NMC - 1))
                GTsb = sc32.tile([65, 65], F32R, name="GTsb", tag="GTsb")
                nc.vector.tensor_copy(GTsb[:], GTps[:])
                # H = G @ wke = GT^T @ wke  [65ci, 512]
                Hps = psg.tile([65, 512], F32, name="Hps", tag="Hps")
                nc.tensor.matmul(Hps[:], GTsb[:], wke, start=True, stop=True)
                Hsb = sc32.tile([65, 512], F32R, name="Hsb", tag="Hsb")
                nc.scalar.copy(Hsb[:], Hps[:])

            # scores chunks + exp + normalize
            attnb = sc32.tile([128, CN * NCH], F32R, name="attnb",
                              tag="attnb")
            rowsum = small.tile([128, 4], F32, name="rowsum", tag="rowsum")
            recip = small.tile([128, 4], F32, name="recip", tag="recip")
            with tc.tile_pool(name="ps_sc", bufs=1, space="PSUM") as pssc:
                sc = [pssc.tile([128, 512], F32, name=f"sc{cc}",
                                tag=f"sc{cc}") for cc in range(NCH)]
                for cc in range(NCH):
                    nc.tensor.matmul(sc[cc][:],
                                     wqe[:, 128 * cc:128 * (cc + 1)],
                                     Hsb[:], start=True, stop=True)
                    nc.scalar.activation(attnb[:, 512 * cc:512 * (cc + 1)],
                                         sc[cc][:], AF.Exp,
                                         bias=0.0, scale=INV_SCALE,
                                         accum_out=rowsum[:, cc:cc + 1])
                # Lrelu table load hidden behind X2/stats work; gated on
                # rowsum so the scheduler cannot hoist it before the exps
                nc.scalar.activation(dummy[:], rowsum[:, 3:4], AF.Lrelu,
                                     bias=0.0, scale=1.0, alpha=SLOPE)
                nc.vector.reciprocal(recip[:], rowsum[:])
                for cc in range(NCH):
                    nc.vector.tensor_scalar_mul(
                        attnb[:, 512 * cc:512 * (cc + 1)],
                        attnb[:, 512 * cc:512 * (cc + 1)],
                        recip[:, cc:cc + 1])

            # aT via PE transposes: block j holds d in [128j, 128j+128)
            with (
                tc.tile_pool(name="tps", bufs=2, space="PSUM") as tps,
                tc.tile_pool(name="ps_a2", bufs=1, space="PSUM") as psa2,
            ):
                for j in range(NCH):
                    tp = tps.tile([128, 512], F32R, name="tp", tag="tp")
                    for cc in range(NCH):
                        nc.tensor.transpose(
                            tp[:, 128 * cc:128 * (cc + 1)],
                            attnb[:, 512 * cc + 128 * j:
                                   512 * cc + 128 * (j + 1)],
                            ident)
                    if j % 2 == 0:
                        nc.vector.tensor_copy(
                            aTall[:, 512 * j:512 * (j + 1)], tp[:])
                    else:
                        nc.scalar.copy(
                            aTall[:, 512 * j:512 * (j + 1)], tp[:])

                # A2 = wv_ext^T @ attn^T  [65, 512]
                A2ps = psa2.tile([65, 512], F32, name="A2ps", tag="A2ps")
                for j in range(NCH):
                    nc.tensor.matmul(A2ps[:], wvt[j],
                                     aTall[:, 512 * j:512 * (j + 1)],
                                     start=(j == 0), stop=(j == NCH - 1))
                nc.scalar.copy(A2sb[:], A2ps[:])

            if dbg is not None:
                nc.sync.dma_start(dbg["attnb"][:], attnb[:])

        if dbg is not None:
            nc.sync.dma_start(dbg["aTall"][:], aTall[:])
            nc.sync.dma_start(dbg["A2sb"][:], A2sb[:])

        # ============ phase 2: X (BN layout) + stats + AllReduce =========
        vre = vsb[:, :].rearrange("ci (cp r) -> ci r cp", r=8)
        with (
            tc.tile_pool(name="stp", bufs=1) as stp,
            tc.tile_pool(name="junkp", bufs=2) as junkp,
            tc.tile_pool(name="cdram", bufs=1, space="DRAM") as cdram,
            tc.tile_pool(name="ps_x", bufs=3, space="PSUM") as psx,
        ):
            pssum = stp.tile([128, 16], F32, name="pssum", tag="pssum")
            pssq = stp.tile([128, 16], F32, name="pssq", tag="pssq")
            red = stp.tile([128, 8], F32, name="red", tag="red")
            for cc in range(NCH):
                for rp in range(4):
                    xt = psx.tile([128, 1024], F32, name="xt", tag="xt")
                    for h in range(2):
                        r = 2 * rp + h
                        nc.tensor.matmul(
                            xt[:, 512 * h:512 * (h + 1)],
                            vre[:, r, 128 * cc:128 * (cc + 1)],
                            A2sb[:], start=True, stop=True)
                    slot = 4 * cc + rp
                    xsl = slice(1024 * rp, 1024 * (rp + 1))
                    nc.scalar.activation(X[cc][:, xsl], xt[:], AF.Copy,
                                         accum_out=pssum[:, slot:slot + 1])
                    junk = junkp.tile([128, 1024], BF16, name="junk",
                                      tag="junk")
                    nc.vector.scalar_tensor_tensor(
                        junk[:], X[cc][:, xsl], 1.0, X[cc][:, xsl],
                        op0=ALU.bypass, op1=ALU.mult,
                        accum_out=pssq[:, slot:slot + 1])
            for cc in range(NCH):
                nc.vector.reduce_sum(red[:, 2 * cc:2 * cc + 1],
                                     pssum[:, 4 * cc:4 * (cc + 1)], axis=AX.X)
                nc.vector.reduce_sum(red[:, 2 * cc + 1:2 * cc + 2],
                                     pssq[:, 4 * cc:4 * (cc + 1)], axis=AX.X)

            # AllGather of per-core stats (half the mesh latency of an
            # AllReduce), then a local strided reduction over cores
            cin = cdram.tile([128, 8], F32, name="cin", tag="cin")
            cout = cdram.tile([8 * 128, 8], F32, name="cout", tag="cout")
            nc.sync.dma_start(cin[:], red[:])
            nc.gpsimd.collective_compute(
                "AllGather", ALU.bypass, replica_groups=RG,
                ins=[cin.opt()], outs=[cout.opt()])
            gath = stp.tile([128, 64], F32, name="gath", tag="gath")
            nc.sync.dma_start(
                gath[:, :].rearrange("p (c s) -> p c s", c=8),
                cout[:, :].rearrange("(c p) s -> p c s", p=128))
            ar = stp.tile([128, 8], F32, name="ar", tag="ar")
            nc.vector.reduce_sum(
                ar[:], gath[:, :].rearrange("p (c s) -> p s c", c=8),
                axis=AX.X)

            # BN affine params; rstd = (var+eps)^-1/2 via bit-trick seed +
            # 2 Newton steps, all on DVE (no ACT table switch needed)
            mean = stp.tile([128, 4], F32, name="mean", tag="mean")
            var = stp.tile([128, 4], F32, name="var", tag="var")
            rstd = stp.tile([128, 4], F32, name="rstd", tag="rstd")
            half = stp.tile([128, 4], F32, name="half", tag="half")
            tmp = stp.tile([128, 4], F32, name="tmpb", tag="tmpb")
            nc.vector.tensor_scalar_mul(
                mean[:], ar[:, :].rearrange("p (c two) -> p two c", two=2)[:, 0, :],
                INV_BHW)
            nc.vector.tensor_scalar_mul(
                var[:], ar[:, :].rearrange("p (c two) -> p two c", two=2)[:, 1, :],
                INV_BHW)
            nc.vector.tensor_mul(tmp[:], mean[:], mean[:])
            nc.vector.tensor_sub(var[:], var[:], tmp[:])
            nc.vector.tensor_scalar_add(var[:], var[:], EPS)
            I32 = mybir.dt.int32
            nc.vector.tensor_scalar(
                out=tmp[:].bitcast(I32), in0=var[:].bitcast(I32),
                scalar1=1, scalar2=None, op0=ALU.arith_shift_right)
            nc.vector.tensor_scalar(
                out=rstd[:].bitcast(I32), in0=tmp[:].bitcast(I32),
                scalar1=-1, scalar2=0x5f3759df, op0=ALU.mult, op1=ALU.add)
            for _ in range(2):
                nc.vector.tensor_mul(tmp[:], rstd[:], rstd[:])
                nc.vector.tensor_mul(tmp[:], tmp[:], var[:])
                nc.vector.tensor_scalar(
                    out=tmp[:], in0=tmp[:], scalar1=-0.5, scalar2=1.5,
                    op0=ALU.mult, op1=ALU.add)
                nc.vector.tensor_mul(rstd[:], rstd[:], tmp[:])
            nc.vector.tensor_mul(alpha[:], bngsb[:], rstd[:])
            nc.vector.tensor_mul(tmp[:], mean[:], alpha[:])
            nc.vector.tensor_sub(beta[:], bnbsb[:], tmp[:])
            if dbg is not None:
                nc.sync.dma_start(dbg["rstd"][:], rstd[:])

            if dbg is not None:
                for cc in range(NCH):
                    nc.sync.dma_start(dbg[f"X{cc}"][:], X[cc][:])
                nc.sync.dma_start(dbg["red"][:], red[:])
                nc.sync.dma_start(dbg["ar"][:], ar[:])
                nc.sync.dma_start(dbg["ab"][:, 0:4], alpha[:])
                nc.sync.dma_start(dbg["ab"][:, 4:8], beta[:])

            # BN + leaky in place, ms-pair-major so w1 unblocks early.
            # cc 0,1 on ACT (fused Lrelu); cc 2,3 on DVE (affine + max)
            with tc.tile_pool(name="bnj", bufs=2) as bnj:
                for mp in range(4):
                    xsl = slice(1024 * mp, 1024 * (mp + 1))
                    for cc in (0, 1):
                        nc.scalar.activation(X[cc][:, xsl], X[cc][:, xsl],
                                             AF.Lrelu,
                                             bias=beta[:, cc:cc + 1],
                                             scale=alpha[:, cc:cc + 1],
                                             alpha=SLOPE)
                    for cc in (2, 3):
                        bt = bnj.tile([128, 1024], BF16, name="bt", tag="bt")
                        nc.vector.tensor_scalar(
                            out=bt[:], in0=X[cc][:, xsl],
                            scalar1=alpha[:, cc:cc + 1],
                            scalar2=beta[:, cc:cc + 1],
                            op0=ALU.mult, op1=ALU.add)
                        nc.vector.scalar_tensor_tensor(
                            X[cc][:, xsl], bt[:], SLOPE, bt[:],
                            op0=ALU.mult, op1=ALU.max)

        # ================= phase 3: w1 -> leaky -> w2 -> y ===============
        with (
            tc.tile_pool(name="y2", bufs=2) as y2p,
            tc.tile_pool(name="outb", bufs=1) as outp,
            tc.tile_pool(name="wps", bufs=4, space="PSUM") as wps,
            tc.tile_pool(name="w2ps", bufs=2, space="PSUM") as w2ps,
        ):
            osb = outp.tile([64, HW], F32, name="osb", tag="osb")
            for ms in range(NS):
                ssl = slice(512 * ms, 512 * (ms + 1))
                y2t = []
                for oc in range(NCH):
                    wp = wps.tile([128, 512], F32, name="wp", tag="wp")
                    for cc in range(NCH):
                        nc.tensor.matmul(
                            wp[:], w1sb[cc][:, 128 * oc:128 * (oc + 1)],
                            X[cc][:, ssl], start=(cc == 0), stop=(cc == 3))
                    yt = y2p.tile([128, 512], BF16, name=f"y2_{oc}",
                                  tag=f"y2_{oc}")
                    nc.scalar.activation(yt[:], wp[:], AF.Lrelu,
                                         bias=b1sb[:, oc:oc + 1],
                                         scale=1.0, alpha=SLOPE)
                    y2t.append(yt)
                fp = w2ps.tile([64, 512], F32, name="fp", tag="fp")
                for oc in range(NCH):
                    nc.tensor.matmul(fp[:], w2sb[oc], y2t[oc][:],
                                     start=(oc == 0), stop=(oc == 3))
                nc.vector.tensor_scalar_add(osb[:, ssl], fp[:],
                                            b2sb[:, 0:1])
                nc.sync.dma_start(d["y"][:, ssl], osb[:, ssl])


_NC_CACHE = {}


def _build(debug=False):
    key = ("dbg" if debug else "nc")
    if key in _NC_CACHE:
        return _NC_CACHE[key]
    nc = bacc.Bacc(trn_type="TRN2", target_bir_lowering=False, debug=False,
                   enable_asserts=False, num_devices=8)
    d = {}
    d["qt"] = nc.dram_tensor("qt", (128, 32 * 65), BF16,
                             kind="ExternalInput").ap()
    d["kt"] = nc.dram_tensor("kt", (128, 32 * 65), BF16,
                             kind="ExternalInput").ap()
    d["v"] = nc.dram_tensor("v", (65, HW), BF16, kind="ExternalInput").ap()
    d["wqek"] = nc.dram_tensor("wqek", (65, 1024), F32R,
                               kind="ExternalInput").ap()
    d["wmisc"] = nc.dram_tensor("wmisc", (128, WM_N), F32R,
                                kind="ExternalInput").ap()
    d["wpk16"] = nc.dram_tensor("wpk16", (128, W16_N), BF16,
                                kind="ExternalInput").ap()
    d["y"] = nc.dram_tensor("y", (64, HW), F32, kind="ExternalOutput").ap()

    dbg = None
    if debug:
        dbg = {}
        dbg["attnb"] = nc.dram_tensor("dbg_attnb", (128, 2048), F32R,
                                      kind="ExternalOutput").ap()
        dbg["rstd"] = nc.dram_tensor("dbg_rstd", (128, 4), F32,
                                     kind="ExternalOutput").ap()
        dbg["aTall"] = nc.dram_tensor("dbg_aTall", (128, 2048), BF16,
                                      kind="ExternalOutput").ap()
        dbg["A2sb"] = nc.dram_tensor("dbg_A2sb", (65, 512), BF16,
                                     kind="ExternalOutput").ap()
        for cc in range(NCH):
            dbg[f"X{cc}"] = nc.dram_tensor(f"dbg_X{cc}", (128, HW), BF16,
                                           kind="ExternalOutput").ap()
        dbg["red"] = nc.dram_tensor("dbg_red", (128, 8), F32,
                                    kind="ExternalOutput").ap()
        dbg["ar"] = nc.dram_tensor("dbg_ar", (128, 8), F32,
                                   kind="ExternalOutput").ap()
        dbg["ab"] = nc.dram_tensor("dbg_ab", (128, 8), F32,
                                   kind="ExternalOutput").ap()
    with tile.TileContext(nc) as tc:
        _body(tc, nc, d, dbg)
    nc.compile()
    _NC_CACHE[key] = nc
    return nc


def _to_bf16(a):
    import ml_dtypes
    return a.astype(ml_dtypes.bfloat16)


def _run(q, k, v, wq, bq, wk, bk, wv, bv, bn_g, bn_b, w1, b1, w2, b2,
         trace=False, tmpdir=None, debug=False):
    nc = _build(debug)
    f = np.float32
    ones = np.ones((1, HW), f)

    wqek = np.zeros((65, 1024), f)
    wqek[:, 0:512] = np.concatenate([wq.T, bq[None, :]], axis=0)
    wqek[:, 512:1024] = np.concatenate([wk.T, bk[None, :]], axis=0)
    wmisc = np.zeros((128, WM_N), f)
    wmisc[:, WM_ID:WM_ID + 128] = np.eye(128, dtype=f)
    wmisc[:, WM_B1:WM_B1 + 4] = b1.reshape(4, 128).T
    wmisc[0:64, WM_B2] = b2
    wmisc[:, WM_BNG:WM_BNG + 4] = bn_g.reshape(4, 128).T
    wmisc[:, WM_BNB:WM_BNB + 4] = bn_b.reshape(4, 128).T

    wpk16 = np.zeros((128, W16_N), f)
    w1t = w1.T.astype(f)
    for cc in range(4):
        wpk16[:, W16_W1 + 512 * cc:W16_W1 + 512 * (cc + 1)] = \
            w1t[128 * cc:128 * (cc + 1), :]
    w2t = w2.T.astype(f)
    for oc in range(4):
        wpk16[:, W16_W2 + 64 * oc:W16_W2 + 64 * (oc + 1)] = \
            w2t[128 * oc:128 * (oc + 1), :]
    wv_ext = np.concatenate([wv, bv[:, None]], axis=1)  # [512, 65]
    for j in range(4):
        wpk16[:, W16_WVT + 65 * j:W16_WVT + 65 * (j + 1)] = \
            wv_ext[128 * j:128 * (j + 1), :]
    wpk16 = _to_bf16(wpk16)

    shared = {"wqek": wqek, "wmisc": wmisc, "wpk16": wpk16}
    in_maps = []
    for b in range(B):
        m = dict(shared)
        for key, src in (("qt", q), ("kt", k)):
            ext = np.concatenate([src[b].reshape(64, HW), ones], axis=0)
            # pre-tiled transpose: [p, 65*j+ci] = ext[ci, 128j+p]
            m[key] = _to_bf16(
                ext.T.reshape(32, 128, 65).transpose(1, 0, 2).reshape(128, -1))
        m["v"] = _to_bf16(np.concatenate([v[b].reshape(64, HW), ones], axis=0))
        in_maps.append(m)
    res = bass_utils.run_bass_kernel_spmd(
        nc, in_maps, core_ids=list(range(8)), trace=trace, tmpdir=tmpdir)
    out = np.stack([res.results[b]["y"].reshape(C, 64, 64) for b in range(B)])
    return out.astype(np.float32), res


def kernel(q, k, v, wq, bq, wk, bk, wv, bv, bn_g, bn_b, w1, b1, w2, b2):
    out, _ = _run(q, k, v, wq, bq, wk, bk, wv, bv, bn_g, bn_b, w1, b1, w2, b2)
    return out
